# revision 24
# baseline (speedup 1.0000x reference)
"""Single-head causal attention (B=16, T=2048, E=384, H=64) on 8 NeuronCores.

Hand-written Bass/Tile kernel, data-parallel over batch B (2 batches per
core). Per core:

  prep:   x (fp32, HBM) --SWDGE cast--> x_bf16 (HBM) --xbar DMA transpose-->
          xT [e,t] bf16 in SBUF;  W cast to bf16.
  proj:   qkT[f,t] = W_qk.T @ xT   (q rows 0:64, k rows 64:128, zero-padded
          to K=128 in SBUF);  v[t,h] = xT.T @ W_v with a fused ones-column
          (row 64 of the PV output then accumulates the softmax denominator
          for free).
  attn:   scores kept transposed sT[s,t] so no probability transpose is
          needed: for each 128-row s-chunk, matmul into PSUM, one ScalarE
          exp over the live causal window (scale=1/8 fused into the
          activation), causal mask via a single gpsimd affine_select on the
          diagonal 128x128 block, then PV matmuls accumulate [65, 512]
          per 512-wide t-tile (row 64 = denominator).
  final:  PE transpose [65,128]->[128,65], DVE reciprocal + per-partition
          scale, one 512 KB output DMA per batch.

The container's walrus build only accepts ONE semaphore wait per
instruction ("Too many sync wait commands"); _split_waits() moves excess
waits emitted by the Tile scheduler onto injected same-engine NoOps, which
is semantically equivalent (engines execute their stream in order).

Execution goes through bass2jax/PJRT on the 8 axon-tunneled cores with a
cached jitted callable, so repeat calls pay only dispatch.
"""

import numpy as np

B, T, E, H = 16, 2048, 384, 64
N_CORES = 8
B_LOC = B // N_CORES  # batches per core
SC = T // 128  # s-chunks per batch (16)
TT = T // 512  # 512-wide t-tiles per batch (4)

_cache = {}


# ---------------------------------------------------------------------------
# kernel body (traced into a Bass/Tile program)
# ---------------------------------------------------------------------------


def _emit_body(nc, tc, pools, x_ap, w_bf, x_bf_dram, out_ap, ident):
    import concourse.mybir as mybir
    from concourse.bass import ds

    fp32 = mybir.dt.float32
    bf16 = mybir.dt.bfloat16
    persist, sbuf = pools["persist"], pools["sbuf"]

    # ---- cast x to bf16 in DRAM (SWDGE dtype-converting DMA), per batch --
    for b in range(B_LOC):
        rows = ds(b * T, T)
        nc.gpsimd.dma_start(x_bf_dram[rows, :], x_ap[rows, :])

    # ---- xbar DMA transpose: xT[b][c] = x_bf[b*T:(b+1)*T, 128c:128c+128].T
    xT = [
        [persist.tile([128, T], bf16, tag=f"xT_{b}_{c}", name=f"xT_{b}_{c}") for c in range(3)]
        for b in range(B_LOC)
    ]
    for b in range(B_LOC):
        for c in range(3):
            nc.sync.dma_start_transpose(
                xT[b][c][:, :], x_bf_dram[ds(b * T, T), ds(c * 128, 128)]
            )

    # ---- projections ----------------------------------------------------
    qT, kT, v_all = pools["qT"], pools["kT"], pools["v_all"]
    psS, psO = pools["psS"], pools["psO"]

    for b in range(B_LOC):
        ps_qk = psO.tile([128, TT * 512], fp32, tag="o", name="ps_qk")
        for tt in range(TT):
            for c in range(3):
                nc.tensor.matmul(
                    ps_qk[:, ds(tt * 512, 512)],
                    w_bf[:, c, 0:128],
                    xT[b][c][:, ds(tt * 512, 512)],
                    start=(c == 0),
                    stop=(c == 2),
                )
        nc.vector.tensor_copy(qT[b][0:64, :], ps_qk[0:64, :])
        nc.vector.tensor_copy(kT[b][0:64, :], ps_qk[64:128, :])
        ps_v = psS.tile([128, 1024], fp32, tag="s", name="ps_v")
        for g in range(SC):
            for c in range(3):
                nc.tensor.matmul(
                    ps_v[:, ds(g * 64, 64)],
                    xT[b][c][:, ds(g * 128, 128)],
                    w_bf[:, c, 128:192],
                    start=(c == 0),
                    stop=(c == 2),
                )
        nc.vector.tensor_copy(
            v_all[b][:, :, 0:64],
            ps_v.rearrange("p (g h) -> p g h", g=16),
        )

    # ---- attention -------------------------------------------------------
    if True:
        for b in range(B_LOC):
            ps_o = psO.tile([128, TT * 512], fp32, tag="o", name="ps_o")
            for i in range(SC):
                t_lo = 128 * i  # live window = [t_lo, T)
                w_live = T - t_lo
                j0 = i // 4
                e_sb = sbuf.tile([128, T], bf16, tag="e", name="e_sb")
                # scores into up to two [128, 1024] psum tiles, exp each
                off = 0
                while off < w_live:
                    seg = min(1024, w_live - off)
                    ps_s = psS.tile([128, 1024], fp32, tag="s", name="ps_s")
                    # matmul segments must stay within 512-col psum banks
                    # and align with 512-boundaries of the t axis
                    o2 = 0
                    while o2 < seg:
                        t_abs = t_lo + off + o2
                        mseg = min(512 - o2 % 512, seg - o2)
                        nc.tensor.matmul(
                            ps_s[:, ds(o2, mseg)],
                            kT[b][:, ds(128 * i, 128)],
                            qT[b][:, ds(t_abs, mseg)],
                            start=True,
                            stop=True,
                        )
                        o2 += mseg
                    nc.scalar.activation(
                        e_sb[:, ds(off, seg)],
                        ps_s[:, 0:seg],
                        mybir.ActivationFunctionType.Exp,
                        scale=0.125,
                    )
                    off += seg
                # causal mask on the diagonal 128x128 block: keep j >= p
                nc.gpsimd.affine_select(
                    out=e_sb[:, 0:128],
                    in_=e_sb[:, 0:128],
                    compare_op=mybir.AluOpType.is_ge,
                    fill=0.0,
                    base=0,
                    pattern=[[1, 128]],
                    channel_multiplier=-1,
                )
                # PV accumulation: out_T[h(+den), t] += v_aug.T @ e
                for j in range(j0, TT):
                    if j == j0:
                        w1 = 512 * (j0 + 1) - t_lo
                        rhs = e_sb[:, 0:w1]
                        dst = ps_o[0:65, ds(512 * j + 512 - w1, w1)]
                    else:
                        rhs = e_sb[:, ds(512 * j - t_lo, 512)]
                        dst = ps_o[0:65, ds(512 * j, 512)]
                    nc.tensor.matmul(
                        dst,
                        v_all[b][:, i, :],
                        rhs,
                        start=(i == 0),
                        stop=(i == 4 * j + 3),
                    )
            # ---- finalize: transpose, divide by denominator, store ------
            out_sb = persist.tile([128, SC * 64], fp32, tag=f"out_{b}", name=f"out_{b}")
            o_sb = sbuf.tile([65, TT * 512], bf16, tag="osb", name="o_sb")
            nc.vector.tensor_copy(o_sb[:, :], ps_o[0:65, :])
            for j in range(TT):
                ps_t = psS.tile([128, 1024], bf16, tag="s", name="ps_t")
                ps_t4 = ps_t[:, 0:264].rearrange("p (q c) -> p q c", q=4)
                for q in range(4):
                    nc.tensor.transpose(
                        ps_t4[:, q, 0:65],
                        o_sb[:, ds(512 * j + q * 128, 128)],
                        ident[0:65, 0:65],
                    )
                rec = sbuf.tile([128, 4], fp32, tag="rec", name="rec")
                nc.vector.reciprocal(rec[:, :], ps_t4[:, :, 64])
                nc.vector.tensor_tensor(
                    out_sb[:, ds(4 * j * 64, 256)].rearrange(
                        "p (q c) -> p q c", q=4
                    ),
                    ps_t4[:, :, 0:64],
                    rec[:, :, None].to_broadcast((128, 4, 64)),
                    mybir.AluOpType.mult,
                )
            nc.sync.dma_start(
                out_ap.rearrange("(b k p) h -> b p k h", b=B_LOC, p=128)[b],
                out_sb.rearrange("p (k h) -> p k h", k=SC),
            )


def build_nc(repeat=1, split=True):
    import concourse.bass as bass
    import concourse.mybir as mybir
    import concourse.tile as tile
    from concourse.masks import make_identity

    fp32 = mybir.dt.float32
    bf16 = mybir.dt.bfloat16

    nc = bass.Bass("TRN2")
    x_ap = nc.dram_tensor("x", [B_LOC * T, E], fp32, kind="ExternalInput").ap()
    w_ap = nc.dram_tensor("w", [E, 3 * H], fp32, kind="ExternalInput").ap()
    out_ap = nc.dram_tensor("out", [B_LOC * T, H], fp32, kind="ExternalOutput").ap()

    with tile.TileContext(nc) as tc:
        with (
            tc.tile_pool(name="persist", bufs=1) as persist,
            tc.tile_pool(name="sbuf", bufs=3) as sbuf,
            tc.tile_pool(name="dram", bufs=1, space="DRAM") as drampool,
            tc.tile_pool(name="psS", bufs=2, space="PSUM") as psS,
            tc.tile_pool(name="psO", bufs=1, space="PSUM") as psO,
        ):
            x_bf_dram = drampool.tile([B_LOC * T, E], bf16, name="x_bf")
            # W: [384, 192] -> SBUF [128, 3, 192] fp32 -> bf16
            w_f32 = persist.tile([128, 3, 192], fp32, tag="wf", name="w_f32")
            nc.sync.dma_start(
                w_f32[:, :, :], w_ap.rearrange("(c p) f -> p c f", p=128)
            )
            w_bf = persist.tile([128, 3, 192], bf16, tag="wb", name="w_bf")
            nc.vector.tensor_copy(w_bf[:, :, :], w_f32[:, :, :])
            ident = persist.tile([128, 128], bf16, tag="ident", name="ident")
            make_identity(nc, ident[:, :])
            # persistent per-batch tensors; zero-pad / ones are set up once
            # (iterations only overwrite the data rows/cols)
            qT = [
                persist.tile([128, T], bf16, tag=f"qT_{b}", name=f"qT_{b}")
                for b in range(B_LOC)
            ]
            kT = [
                persist.tile([128, T], bf16, tag=f"kT_{b}", name=f"kT_{b}")
                for b in range(B_LOC)
            ]
            v_all = [
                persist.tile([128, SC, 65], bf16, tag=f"v_{b}", name=f"v_{b}")
                for b in range(B_LOC)
            ]
            for b in range(B_LOC):
                # rows 64:128 stay zero so score matmuls contract over K=128
                nc.vector.memset(qT[b][64:128, :], 0.0)
                nc.vector.memset(kT[b][64:128, :], 0.0)
                # col 64 of each v chunk stays 1.0 (softmax denominator row)
                nc.vector.memset(v_all[b][:, :, 64:65], 1.0)
            pools = {
                "persist": persist,
                "sbuf": sbuf,
                "psS": psS,
                "psO": psO,
                "qT": qT,
                "kT": kT,
                "v_all": v_all,
            }
            for _ in range(repeat):
                _emit_body(nc, tc, pools, x_ap, w_bf, x_bf_dram, out_ap, ident)

    if split:
        _split_waits(nc)
    return nc


# ---------------------------------------------------------------------------
# walrus workaround: at most one semaphore wait per instruction
# ---------------------------------------------------------------------------


def _split_waits(nc, maxw=1):
    import concourse.mybir as mybir

    nsplit = 0
    for fn in nc.m.functions:
        for blk in fn.blocks:
            new_insts = []
            for inst in blk.instructions:
                si = inst.sync_info
                if si is not None and si.on_wait is not None and len(si.on_wait) > maxw:
                    waits = list(si.on_wait)
                    while len(waits) > maxw:
                        chunk, waits = waits[:maxw], waits[maxw:]
                        nop = mybir.InstNoOp(name=f"waitsplit_{nsplit}", ins=[], outs=[])
                        nop.engine = inst.engine
                        nop.sync_info = mybir.SyncInfo(on_wait=chunk, on_update=[])
                        new_insts.append(nop)
                        nsplit += 1
                    inst.sync_info = mybir.SyncInfo(
                        on_wait=waits, on_update=list(si.on_update or [])
                    )
                new_insts.append(inst)
            blk.instructions = new_insts
    return nsplit


# ---------------------------------------------------------------------------
# execution via bass2jax / PJRT on the 8 axon-tunneled cores
# ---------------------------------------------------------------------------


def _build_runner(nc, n_cores=N_CORES, donate=True):
    import jax

    try:
        from jax.experimental.shard_map import shard_map
    except Exception:
        from jax import shard_map
    from jax.sharding import Mesh, PartitionSpec

    import concourse.mybir as mybir
    from concourse import bass2jax
    from concourse.bass2jax import _bass_exec_p, install_neuronx_cc_hook

    install_neuronx_cc_hook()

    in_names, out_names, out_avals = [], [], []
    for alloc in nc.m.functions[0].allocations:
        if not isinstance(alloc, mybir.MemoryLocationSet):
            continue
        name = alloc.memorylocations[0].name
        if alloc.kind == "ExternalInput":
            if nc.partition_id_tensor is None or name != nc.partition_id_tensor.name:
                in_names.append(name)
        elif alloc.kind == "ExternalOutput":
            out_names.append(name)
            out_avals.append(
                jax.core.ShapedArray(tuple(alloc.tensor_shape), mybir.dt.np(alloc.dtype))
            )
    n_params = len(in_names)
    n_outs = len(out_avals)
    all_in_names = list(in_names) + list(out_names)
    if nc.partition_id_tensor is not None:
        all_in_names.append(nc.partition_id_tensor.name)

    def _body(*args):
        operands = list(args)
        if nc.partition_id_tensor is not None:
            operands.append(bass2jax.partition_id_tensor())
        outs = _bass_exec_p.bind(
            *operands,
            out_avals=tuple(out_avals),
            in_names=tuple(all_in_names),
            out_names=tuple(out_names),
            lowering_input_output_aliases=(),
            sim_require_finite=True,
            sim_require_nnan=True,
            nc=nc,
        )
        return tuple(outs)

    devices = jax.devices()[:n_cores]
    mesh = Mesh(np.asarray(devices), ("core",))
    donate_nums = tuple(range(n_params, n_params + n_outs)) if donate else ()
    sharded = jax.jit(
        shard_map(
            _body,
            mesh=mesh,
            in_specs=(PartitionSpec("core"),) * (n_params + n_outs),
            out_specs=(PartitionSpec("core"),) * n_outs,
            check_rep=False,
        ),
        donate_argnums=donate_nums,
        keep_unused=True,
    )

    zero_shapes = [(a.shape, a.dtype) for a in out_avals]

    def run(in_maps):
        concat_in = [
            np.concatenate([np.asarray(m[name]) for m in in_maps], axis=0)
            for name in in_names
        ]
        concat_zeros = [np.zeros((n_cores * s[0], *s[1:]), d) for (s, d) in zero_shapes]
        out_arrs = sharded(*concat_in, *concat_zeros)
        outs = [np.asarray(a) for a in out_arrs]
        return [
            {
                name: outs[i].reshape(n_cores, *out_avals[i].shape)[c]
                for i, name in enumerate(out_names)
            }
            for c in range(n_cores)
        ]

    run.sharded = sharded
    run.in_names = in_names
    run.zero_shapes = zero_shapes
    run.mesh = mesh
    return run


def get_runner(repeat=1, donate=True):
    key = ("runner", repeat, donate)
    if key not in _cache:
        nc = build_nc(repeat)
        _cache[key] = _build_runner(nc, donate=donate)
    return _cache[key]


def kernel(x: np.ndarray, W_qkv: np.ndarray) -> np.ndarray:
    run = get_runner(1)
    x = np.ascontiguousarray(x, dtype=np.float32).reshape(N_CORES, B_LOC * T, E)
    w = np.ascontiguousarray(W_qkv, dtype=np.float32)
    in_maps = [{"x": x[c], "w": w} for c in range(N_CORES)]
    res = run(in_maps)
    out = np.stack([res[c]["out"] for c in range(N_CORES)])
    return out.reshape(B, T, H)


if __name__ == "__main__":
    rng = np.random.default_rng(0)
    x = rng.standard_normal((B, T, E), dtype=np.float32)
    W = (rng.standard_normal((E, 3 * H)) * (E**-0.5)).astype(np.float32)
    out = kernel(x=x, W_qkv=W)
    print("out", out.shape, out.dtype, float(np.abs(out).max()))
